# revision 67
# baseline (speedup 1.0000x reference)
"""BiLSTM classifier Trainium2 kernel (8 NeuronCores, SPMD).

Model (reference): emb = table[x]; c_f = LSTM_final_cell(emb, fwd);
c_b = LSTM_final_cell(flip(emb), bwd); out = [c_f, c_b] @ Wd + bd.

Sharding: 8 cores = 2 directions x 4 batch-shards of 64 rows; each core runs
2 independent LSTM "chains" of batch 32, software-pipelined HALF A STEP apart
so each chain's serial step latency (the wall-clock limiter: PE -> sigmoid ->
c-update -> tanh -> h-update, ~2.17us/step in the timeline model) overlaps
the other chain's engine time instead of serializing with it. All state is
TRANSPOSED on-chip: gates/hidden on partitions, batch along the free dim.

Per step (per chain), z^T accumulates in one PSUM bank laid out
[i i f f g g o o] (B cols per 128-gate block):
  z^T = Wx[m]^T @ emb_t^T   (8 matmuls, no h dependency -> issued one round
                             early, during the previous step)
      + Wh[k,m]^T @ h^T[k]  (16 matmuls; only the 12 i,f,g ones gate the
                             critical path; o's run after)
then
  sg = sigmoid(z_ifg)            (one Act op; g pre-doubled via host fold so
                                  sg_g = (tanh(z_g)+1)/2)
  so = sigmoid(z_o)              (Act, off critical path, bf16)
  t2 = (sg_g-0.5)*sg_i ; t1 = sg_f*c ; c = 2*t2 + t1   (DVE stt/tt/stt)
  tc = tanh(c)                   (Act, bf16)
  h  = so * tc                   (DVE tensor-tensor, all-bf16 2x mode)
The last step emits only the c-path (o/tanh/h are dead there).

emb^T comes from an indirect-DMA gather of embedding rows (128 tokens/instr,
schedule precomputed on host; the full index table is preloaded once) + PE
transpose + copy. Gather DMAs are launched early in the PRIOR iteration and
the transposes/copies deferred until the data is long since landed, with one
embT tile per (chain, 128-token slice) so overwrite WARs release per-slice
and never collide with the iteration boundary. Final: partial logits
(4 x 32) = Wd_half^T @ c per chain, summed across direction pairs on host.
"""

import sys

for _p in ("/root/.axon_site/_ro/trn_rl_repo", "/opt/trn_rl_repo"):
    if _p not in sys.path:
        sys.path.insert(0, _p)

import numpy as np
import ml_dtypes

# ---- problem constants (hardcoded; kernel.py must be self-contained) ----
VOCAB = 32000
EMBED = 128
HIDDEN = 256
NUM_CLASSES = 4
B_FULL, T_FULL = 256, 512

import os
N_CORES = 8
CHAINS = int(os.environ.get("KNOB_CHAINS", "2"))
B = 64 // CHAINS    # batch per chain
STEPS = 16          # time steps per iteration block
N_ITERS = T_FULL // STEPS
GB = 8 * B          # gate-row block per step in z^T layout ( = 4H/128 * B )
TPC = STEPS * B // 128      # gather tiles per chain per iteration
W_NP = ml_dtypes.bfloat16   # on-chip matmul operand dtype
SEQV = os.environ.get("KNOB_SEQ", "offset")   # emission-order variant
WIDE_GATHER = os.environ.get("KNOB_WIDE_GATHER", "0") == "1"
# multi-index gathers (WIDE) and bf16 gathers mis-route data on HW when
# combined with the rest of the pipeline; keep validated f32 single-index
# gathers by default.
GATHER_BF16 = (os.environ.get("KNOB_GATHER_BF16", "0") == "1"
               and not WIDE_GATHER)
G_NP = ml_dtypes.bfloat16 if GATHER_BF16 else np.float32

_CACHE = {}


def _build_program(with_bias=True):
    import concourse.bacc as bacc
    import concourse.mybir as mybir
    from concourse import bass
    from concourse.tile import TileContext

    f32 = mybir.dt.float32
    i32 = mybir.dt.int32
    wdt = mybir.dt.bfloat16
    SIG = mybir.ActivationFunctionType.Sigmoid
    TANH = mybir.ActivationFunctionType.Tanh
    MULT = mybir.AluOpType.mult
    ADD = mybir.AluOpType.add
    SUB = mybir.AluOpType.subtract

    nc = bacc.Bacc("TRN2", target_bir_lowering=False, debug=False,
                   num_devices=N_CORES,
                   dynamic_dma_scratch_size=int(os.environ.get(
                       "KNOB_DMA_SCRATCH", "16384")))

    # ---- DRAM I/O ----
    gdt = wdt if GATHER_BF16 else f32
    emb_dram = nc.dram_tensor("emb", [VOCAB, EMBED], gdt,
                              kind="ExternalInput")
    # 24 stationary tiles per gate-chunk m: (m, k<2) = Wh block, (m, 2) = Wx
    whx_dram = nc.dram_tensor("whxT", [128, 24 * 128], wdt,
                              kind="ExternalInput")
    bb_dram = nc.dram_tensor("bbT", [128, GB], wdt, kind="ExternalInput")
    wdT_dram = nc.dram_tensor("wdT", [128, 8], f32, kind="ExternalInput")
    idf_dram = nc.dram_tensor("identf", [128, 128], f32, kind="ExternalInput")
    idw_dram = nc.dram_tensor("identw", [128, 128], wdt, kind="ExternalInput")
    idx_dram = nc.dram_tensor("idx", [128, N_ITERS * CHAINS * TPC], i32,
                              kind="ExternalInput")
    out_dram = nc.dram_tensor("out", [CHAINS, NUM_CLASSES, B], f32,
                              kind="ExternalOutput")

    with TileContext(nc) as tc:
        with (
            tc.tile_pool(name="const", bufs=1) as constp,
            tc.tile_pool(name="state", bufs=1) as statep,

            tc.tile_pool(name="embp", bufs=8) as embp,
            tc.tile_pool(name="idxwp", bufs=3) as idxwp,
            tc.tile_pool(name="embTp", bufs=3) as embTp,
            tc.tile_pool(name="sgp", bufs=2) as sgp,
            tc.tile_pool(name="sop", bufs=2) as sop,
            tc.tile_pool(name="tmpp", bufs=2) as tmpp,
            tc.tile_pool(name="outp", bufs=1) as outp,
            tc.tile_pool(name="zps0", bufs=(1 if SEQV == "quad" else 2),
                         space="PSUM") as zps0,
            tc.tile_pool(name="zps1", bufs=(1 if SEQV == "quad" else 2),
                         space="PSUM") as zps1,
            tc.tile_pool(name="trps", bufs=2, space="PSUM") as trps,
            tc.tile_pool(name="dps", bufs=1, space="PSUM") as dps,
        ):
            zps = [zps0, zps1]

            # ---- load constants ----
            whx = constp.tile([128, 24 * 128], wdt)
            bb = constp.tile([128, GB], wdt)
            wdT = constp.tile([128, 8], f32)
            idf = constp.tile([128, 128], f32)
            idw = constp.tile([128, 128], wdt)
            idx_sb = constp.tile([128, N_ITERS * CHAINS * TPC], i32,
                                 name="idx_sb")
            # idx gates the first gather and idf the first transpose; load
            # them before the large whx tensor so the prologue overlaps.
            for dst, src in ((idx_sb, idx_dram), (idf, idf_dram),
                             (whx, whx_dram), (bb, bb_dram),
                             (wdT, wdT_dram), (idw, idw_dram)):
                nc.sync.dma_start(out=dst[:], in_=src[:])

            # ---- per-chain persistent state ----
            hT = [statep.tile([128, 2 * B], wdt, tag=f"hT{c}",
                              name=f"hT{c}") for c in range(CHAINS)]
            cst = [statep.tile([128, 2 * B], f32, tag=f"c{c}",
                               name=f"cst{c}") for c in range(CHAINS)]
            for c in range(CHAINS):
                nc.vector.memset(hT[c][:], 0.0)
                nc.vector.memset(cst[c][:], 0.0)

            def emit_precompute(it):
                """Gather + transpose emb block for iteration `it`.
                Returns (dma_units, xf_units, embT tiles): DMA launches are
                emitted early in the iteration; the PE transposes + copies
                are deferred until the gathers are surely complete so they
                never head-of-line-block the recurrence matmuls."""
                dma_units, xf_units = [], []
                # one tile per (chain, slice): WAR on an overwrite releases
                # as soon as that slice's last x-projection read retires,
                # instead of waiting for the whole iteration's reads.
                embTs = [[embTp.tile([128, 128], wdt, tag=f"embT{c}{j}",
                                     name=f"embT{c}{j}")
                          for j in range(TPC)] for c in range(CHAINS)]
                ets = {}
                base = it * CHAINS * TPC
                if WIDE_GATHER:
                    for c in range(CHAINS):
                        def g_unit(c=c):
                            # ucode requires a packed offset-0 index AP:
                            # repack this (iter, chain)'s columns first.
                            idxw = idxwp.tile([128, TPC], i32,
                                              tag=f"idxw{c}", name=f"idxw{c}")
                            nc.vector.tensor_copy(
                                out=idxw[:],
                                in_=idx_sb[:, base + c * TPC:
                                           base + (c + 1) * TPC])
                            et = embp.tile([128, TPC * 128], gdt,
                                           tag=f"emb{c}", name=f"emb{c}")
                            for j in range(TPC):
                                ets[(c, j)] = et[:, j * 128:(j + 1) * 128]
                            nc.gpsimd.indirect_dma_start(
                                out=et[:], out_offset=None, in_=emb_dram[:],
                                in_offset=bass.IndirectOffsetOnAxis(
                                    ap=idxw[:], axis=0))
                        dma_units.append(g_unit)
                    for j in range(TPC):
                        for c in range(CHAINS):
                            def x_unit(c=c, j=j):
                                tp = trps.tile([128, 128], gdt, name="tp")
                                nc.tensor.transpose(
                                    out=tp[:], in_=ets[(c, j)],
                                    identity=(idw[:] if GATHER_BF16
                                              else idf[:]))
                                nc.vector.tensor_copy(
                                    out=embTs[c][j][:], in_=tp[:])
                            xf_units.append(x_unit)
                    return dma_units, xf_units, embTs
                for j in range(TPC):
                    for c in range(CHAINS):
                        def g_unit(c=c, j=j):
                            et = embp.tile([128, 128], gdt, tag=f"emb{c}{j}",
                                           name=f"emb{c}{j}")
                            ets[(c, j)] = et
                            nc.gpsimd.indirect_dma_start(
                                out=et[:], out_offset=None, in_=emb_dram[:],
                                in_offset=bass.IndirectOffsetOnAxis(
                                    ap=idx_sb[:, base + c * TPC + j:
                                              base + c * TPC + j + 1],
                                    axis=0))
                        def x_unit(c=c, j=j):
                            tp = trps.tile([128, 128], gdt, name="tp")
                            nc.tensor.transpose(
                                out=tp[:], in_=ets[(c, j)][:],
                                identity=(idw[:] if GATHER_BF16
                                          else idf[:]))
                            nc.vector.tensor_copy(
                                out=embTs[c][j][:], in_=tp[:])
                        dma_units.append(g_unit)
                        xf_units.append(x_unit)
                return dma_units, xf_units, embTs

            # ---- pipeline state ----
            ztile = [None] * CHAINS        # PSUM z for the in-flight step
            sgt = [None] * CHAINS
            sot = [None] * CHAINS
            tct = [None] * CHAINS

            def zsl(c, m):
                """column slice of z for gate-block m (0..7)."""
                return ztile[c][:, m * B:(m + 1) * B]

            def emit_x(c, s, embT_c):
                """Create step-s PSUM tile; bias + 8 emb-projection matmuls."""
                ztile[c] = zps[c % 2].tile([128, 8 * B], f32, tag=f"z{c}",
                                           name=f"z{c}")
                if with_bias:
                    nc.tensor.matmul(
                        out=ztile[c][:], lhsT=idw[:], rhs=bb[:],
                        start=True, stop=False, skip_group_check=True)
                sl = s % STEPS
                j, jo = sl * B // 128, (sl * B) % 128
                emb_s = embT_c[j][:, jo:jo + B]
                for m in range(8):
                    nc.tensor.matmul(
                        out=zsl(c, m),
                        lhsT=whx[:, (m * 3 + 2) * 128:(m * 3 + 3) * 128],
                        rhs=emb_s,
                        start=(not with_bias and m == 0),
                        stop=False, skip_group_check=True)

            def emit_ifg_mms(c, hi=6):
                for k in range(2):
                    for m in range(hi):
                        nc.tensor.matmul(
                            out=zsl(c, m),
                            lhsT=whx[:, (m * 3 + k) * 128:
                                     (m * 3 + k + 1) * 128],
                            rhs=hT[c][:, k * B:(k + 1) * B],
                            start=False, stop=(k == 1 and m == hi - 1),
                            skip_group_check=True)

            def emit_o_mms(c):
                for k in range(2):
                    for m in range(6, 8):
                        nc.tensor.matmul(
                            out=zsl(c, m),
                            lhsT=whx[:, (m * 3 + k) * 128:
                                     (m * 3 + k + 1) * 128],
                            rhs=hT[c][:, k * B:(k + 1) * B],
                            start=False, stop=(k == 1 and m == 7),
                            skip_group_check=True)

            def emit_sig(c, hi=6):
                sg = sgp.tile([128, hi * B], f32, tag=f"sg{c}",
                              name=f"sg{c}")
                sgt[c] = sg
                nc.scalar.activation(out=sg[:], in_=ztile[c][:, 0:hi * B],
                                     func=SIG)

            def emit_sigo(c):
                so = sop.tile([128, 2 * B], wdt, tag=f"so{c}", name=f"so{c}")
                sot[c] = so
                nc.scalar.activation(out=so[:], in_=ztile[c][:, 6 * B:8 * B],
                                     func=SIG)

            def emit_cupd(c):
                sg = sgt[c]
                t2 = tmpp.tile([128, 2 * B], f32, tag=f"t2{c}", name=f"t2{c}")
                t1 = tmpp.tile([128, 2 * B], f32, tag=f"t1{c}", name=f"t1{c}")
                # t2 = (sig_g-0.5)*sig_i ; t1 = sig_f*c ; c = 2*t2 + t1
                nc.vector.scalar_tensor_tensor(
                    out=t2[:], in0=sg[:, 4 * B:6 * B], scalar=0.5,
                    in1=sg[:, 0:2 * B], op0=SUB, op1=MULT)
                nc.vector.tensor_mul(
                    out=t1[:], in0=sg[:, 2 * B:4 * B], in1=cst[c][:])
                nc.vector.scalar_tensor_tensor(
                    out=cst[c][:], in0=t2[:], scalar=2.0,
                    in1=t1[:], op0=MULT, op1=ADD)

            def emit_tanh(c):
                tcc = tmpp.tile([128, 2 * B], wdt, tag=f"tc{c}",
                                name=f"tc{c}")
                tct[c] = tcc
                nc.scalar.activation(out=tcc[:], in_=cst[c][:], func=TANH)

            def emit_h(c):
                # h = sigmoid(z_o) * tanh(c)   (all-bf16 tensor-tensor, 2x)
                so = sot[c][:] if sot[c] is not None else \
                    sgt[c][:, 6 * B:8 * B]
                nc.vector.tensor_mul(out=hT[c][:], in0=so,
                                     in1=tct[c][:])

            # ---- prologue: first gathers + step-0 x-projections ----
            # launch the j<=1 gathers, transpose j0, start step 0; the
            # rest of iteration 0's and all of iteration 1's prefetch
            # drains inside the first rounds (the loop prefetches two
            # iterations ahead from then on).
            dma_u, xf_u, embT = emit_precompute(0)
            for u in dma_u[:2 * CHAINS]:
                u()
            for u in xf_u[:CHAINS]:
                u()
            for c in range(CHAINS):
                emit_x(c, 0, embT[c])
            for u in xf_u[CHAINS:2 * CHAINS]:
                u()
            dma_u = dma_u[2 * CHAINS:]
            xf_u = xf_u[2 * CHAINS:]
            dma_u1, xf_u1, embT_nxt = emit_precompute(1)
            dma_u += dma_u1
            xf_u += xf_u1
            dma_nx, xf_nx = [], []
            embT_nx2 = None

            A, Bc = 0, 1
            for r in range(T_FULL):
                it = r // STEPS
                if r % STEPS == 0 and it + 2 < N_ITERS:
                    dma_nx, xf_nx, embT_nx2 = emit_precompute(it + 2)
                if (r + 1) % STEPS == 0:
                    # next round's x-projections read embT_nxt: flush all
                    # remaining gather/transpose writes before those reads.
                    while dma_u:
                        dma_u.pop(0)()
                    while xf_u:
                        xf_u.pop(0)()
                    dma_u, dma_nx = dma_nx, []
                    xf_u, xf_nx = xf_nx, []

                def x_next(c):
                    if r + 1 >= T_FULL:
                        return
                    src = embT_nxt if (r + 1) % STEPS == 0 else embT
                    emit_x(c, r + 1, src[c])

                if SEQV == "quad":
                    # 4 chains at quarter-round offsets, merged sigma(ifgo)
                    for c, ph in ((0, 1), (3, 2), (1, 1), (0, 2),
                                  (2, 1), (1, 2), (3, 1), (2, 2)):
                        if ph == 1:
                            sot[c] = None
                            emit_ifg_mms(c, hi=8)
                            emit_sig(c, hi=8)
                            x_next(c)
                        else:
                            if r == 0 and c == 3:
                                continue
                            emit_cupd(c); emit_tanh(c); emit_h(c)
                elif SEQV == "lockstep":
                    emit_ifg_mms(A); emit_sig(A)
                    emit_ifg_mms(Bc); emit_sig(Bc)
                    emit_o_mms(A); emit_sigo(A); x_next(A)
                    emit_o_mms(Bc); emit_sigo(Bc); x_next(Bc)
                    emit_cupd(A); emit_tanh(A); emit_h(A)
                    emit_cupd(Bc); emit_tanh(Bc); emit_h(Bc)
                else:
                    # half-step offset: A runs step r in the first half of
                    # the round, B's step r-1 back-half completes, then B
                    # starts step r in the second half. The last step only
                    # needs c (the model output), so its o/tanh/h are dead.
                    last = r == T_FULL - 1
                    emit_ifg_mms(A); emit_sig(A)
                    if r > 0:
                        emit_cupd(Bc)          # B's step r-1 c-chain
                        emit_tanh(Bc)
                    if not last:
                        emit_o_mms(A); emit_sigo(A)
                    x_next(A)
                    if r > 0:
                        emit_h(Bc)
                    emit_cupd(A)
                    if not last:
                        emit_tanh(A)
                    emit_ifg_mms(Bc); emit_sig(Bc)
                    if not last:
                        emit_h(A)
                        emit_o_mms(Bc); emit_sigo(Bc)
                    x_next(Bc)

                # spread next iteration's prefetch: DMA launches first
                # (2/round), deferred transposes+copies later (1/round).
                for _ in range(2):
                    if dma_u:
                        dma_u.pop(0)()
                if xf_u and (r >= 4 if it == 0 else
                             (not dma_u and r % STEPS >= 4)):
                    xf_u.pop(0)()
                if r % STEPS == STEPS - 1 and embT_nxt is not None:
                    embT = embT_nxt
                    embT_nxt = embT_nx2
                    embT_nx2 = None

            def emit_epilogue(c):
                # dense epilogue: partial logits = (Wd_half)^T @ c
                dp = dps.tile([NUM_CLASSES, B], f32, tag=f"dp{c}",
                              name=f"dp{c}")
                for k in range(2):
                    nc.tensor.matmul(
                        out=dp[:], lhsT=wdT[:, k * 4:(k + 1) * 4],
                        rhs=cst[c][:, k * B:(k + 1) * B],
                        start=(k == 0), stop=(k == 1))
                ob = outp.tile([NUM_CLASSES, B], f32, tag=f"ob{c}",
                               name=f"ob{c}")
                nc.vector.tensor_copy(out=ob[:], in_=dp[:])
                nc.sync.dma_start(out=out_dram[c], in_=ob[:])

            if SEQV == "quad":
                emit_cupd(3)
                for c in range(CHAINS):
                    emit_epilogue(c)
            elif SEQV != "lockstep":
                emit_epilogue(A)
                emit_cupd(Bc)
                emit_epilogue(Bc)
            else:
                for c in range(CHAINS):
                    emit_epilogue(c)

    nc.compile()
    return nc


def _prep_core_inputs(core, x, emb_np, Wx, Wh, b, Wd):
    """Host-side prep: weight layout/scaling + gather index schedule."""
    d, s = core // 4, core % 4
    Wx = Wx.astype(np.float32).copy()
    Wh = Wh.astype(np.float32).copy()
    b = b.astype(np.float32).copy()
    # fold tanh(z_g) = 2*sigmoid(2 z_g) - 1: double the g-gate inputs.
    Wx[:, 512:768] *= 2.0
    b[512:768] *= 2.0
    Wh[:, 512:768] *= 2.0

    whx = np.empty((128, 24 * 128), np.float32)
    for m in range(8):
        for k in range(2):
            whx[:, (m * 3 + k) * 128:(m * 3 + k + 1) * 128] = \
                Wh[k * 128:(k + 1) * 128, m * 128:(m + 1) * 128]
        whx[:, (m * 3 + 2) * 128:(m * 3 + 3) * 128] = \
            Wx[:, m * 128:(m + 1) * 128]
    bb = np.repeat(b.reshape(8, 128).T[:, :, None], B, axis=2).reshape(128, GB)
    wdT = np.empty((128, 8), np.float32)
    for k in range(2):
        wdT[:, k * 4:(k + 1) * 4] = Wd[d * 256 + k * 128:
                                       d * 256 + (k + 1) * 128, :]

    it = np.arange(N_ITERS)[:, None, None]
    p = np.arange(128)[None, :, None]
    cj = np.arange(CHAINS * TPC)[None, None, :]
    chain, j = cj // TPC, cj % TPC
    s_local = j * (128 // B) + p // B
    jb = p % B
    t = it * STEPS + s_local
    if d == 1:
        t = (T_FULL - 1) - t
    row = s * 64 + chain * B + jb
    idx = x[row, t].astype(np.int32)          # [N_ITERS, 128, CHAINS*TPC]
    idx = np.ascontiguousarray(idx.transpose(1, 0, 2).reshape(128, -1))

    return {
        "emb": emb_np,
        "whxT": np.ascontiguousarray(whx.astype(W_NP)),
        "bbT": np.ascontiguousarray(bb.astype(W_NP)),
        "wdT": wdT,
        "identf": np.eye(128, dtype=np.float32),
        "identw": np.eye(128).astype(W_NP),
        "idx": idx,
    }


def kernel(x, train, embed_table, Wx_f, Wh_f, b_f, Wx_b, Wh_b, b_b, Wd, bd,
           **_unused):
    from concourse.bass_utils import run_bass_kernel_spmd

    x = np.asarray(x).astype(np.int64)
    emb_np = np.ascontiguousarray(
        np.asarray(embed_table, np.float32).astype(G_NP))
    Wd_np = np.asarray(Wd, np.float32)

    with_bias = bool(np.any(np.asarray(b_f)) or np.any(np.asarray(b_b)))
    key = ("nc", with_bias)
    if key not in _CACHE:
        _CACHE[key] = _build_program(with_bias)
    nc = _CACHE[key]

    in_maps = []
    for core in range(N_CORES):
        if core < 4:
            Wx, Wh, b = Wx_f, Wh_f, b_f
        else:
            Wx, Wh, b = Wx_b, Wh_b, b_b
        in_maps.append(_prep_core_inputs(
            core, x, emb_np, np.asarray(Wx), np.asarray(Wh), np.asarray(b),
            Wd_np))

    res = run_bass_kernel_spmd(nc, in_maps, list(range(N_CORES))).results

    logits = np.zeros((B_FULL, NUM_CLASSES), np.float32)
    for core in range(N_CORES):
        s = core % 4
        o = np.asarray(res[core]["out"], np.float32)  # [CHAINS, 4, B]
        for c in range(CHAINS):
            r0 = s * 64 + c * B
            logits[r0:r0 + B] += o[c].T
    logits += np.asarray(bd, np.float32)[None, :]
    return logits


# revision 75
# speedup vs baseline: 1.0003x; 1.0003x over previous
"""BiLSTM classifier Trainium2 kernel (8 NeuronCores, SPMD).

Model (reference): emb = table[x]; c_f = LSTM_final_cell(emb, fwd);
c_b = LSTM_final_cell(flip(emb), bwd); out = [c_f, c_b] @ Wd + bd.

Sharding: 8 cores = 2 directions x 4 batch-shards of 64 rows; each core runs
2 independent LSTM "chains" of batch 32, software-pipelined HALF A STEP apart
so each chain's serial step latency (the wall-clock limiter: PE -> sigmoid ->
c-update -> tanh -> h-update, ~2.17us/step in the timeline model) overlaps
the other chain's engine time instead of serializing with it. All state is
TRANSPOSED on-chip: gates/hidden on partitions, batch along the free dim.

Per step (per chain), z^T accumulates in one PSUM bank laid out
[i i f f g g o o] (B cols per 128-gate block):
  z^T = Wx[m]^T @ emb_t^T   (8 matmuls, no h dependency -> issued one round
                             early, during the previous step)
      + Wh[k,m]^T @ h^T[k]  (16 matmuls; only the 12 i,f,g ones gate the
                             critical path; o's run after)
then
  sg = sigmoid(z_ifg)            (one Act op; g pre-doubled via host fold so
                                  sg_g = (tanh(z_g)+1)/2)
  so = sigmoid(z_o)              (Act, off critical path, bf16)
  t2 = (sg_g-0.5)*sg_i ; t1 = sg_f*c ; c = 2*t2 + t1   (DVE stt/tt/stt)
  tc = tanh(c)                   (Act, bf16)
  h  = so * tc                   (DVE tensor-tensor, all-bf16 2x mode)
The last step emits only the c-path (o/tanh/h are dead there).

emb^T comes from an indirect-DMA gather of embedding rows (128 tokens/instr,
schedule precomputed on host; the full index table is preloaded once) + PE
transpose + copy. Gather DMAs are launched early in the PRIOR iteration and
the transposes/copies deferred until the data is long since landed, with one
embT tile per (chain, 128-token slice) so overwrite WARs release per-slice
and never collide with the iteration boundary. Final: partial logits
(4 x 32) = Wd_half^T @ c per chain, summed across direction pairs on host.
"""

import sys

for _p in ("/root/.axon_site/_ro/trn_rl_repo", "/opt/trn_rl_repo"):
    if _p not in sys.path:
        sys.path.insert(0, _p)

import numpy as np
import ml_dtypes

# ---- problem constants (hardcoded; kernel.py must be self-contained) ----
VOCAB = 32000
EMBED = 128
HIDDEN = 256
NUM_CLASSES = 4
B_FULL, T_FULL = 256, 512

import os
N_CORES = 8
CHAINS = int(os.environ.get("KNOB_CHAINS", "2"))
B = 64 // CHAINS    # batch per chain
STEPS = 16          # time steps per iteration block
N_ITERS = T_FULL // STEPS
GB = 8 * B          # gate-row block per step in z^T layout ( = 4H/128 * B )
TPC = STEPS * B // 128      # gather tiles per chain per iteration
W_NP = ml_dtypes.bfloat16   # on-chip matmul operand dtype
SEQV = os.environ.get("KNOB_SEQ", "offset")   # emission-order variant
WIDE_GATHER = os.environ.get("KNOB_WIDE_GATHER", "0") == "1"
# multi-index gathers (WIDE) and bf16 gathers mis-route data on HW when
# combined with the rest of the pipeline; keep validated f32 single-index
# gathers by default.
GATHER_BF16 = (os.environ.get("KNOB_GATHER_BF16", "0") == "1"
               and not WIDE_GATHER)
G_NP = ml_dtypes.bfloat16 if GATHER_BF16 else np.float32

_CACHE = {}


def _build_program(with_bias=True):
    import concourse.bacc as bacc
    import concourse.mybir as mybir
    from concourse import bass
    from concourse.tile import TileContext

    f32 = mybir.dt.float32
    i32 = mybir.dt.int32
    wdt = mybir.dt.bfloat16
    SIG = mybir.ActivationFunctionType.Sigmoid
    TANH = mybir.ActivationFunctionType.Tanh
    MULT = mybir.AluOpType.mult
    ADD = mybir.AluOpType.add
    SUB = mybir.AluOpType.subtract

    nc = bacc.Bacc("TRN2", target_bir_lowering=False, debug=False,
                   num_devices=N_CORES,
                   dynamic_dma_scratch_size=int(os.environ.get(
                       "KNOB_DMA_SCRATCH", "16384")))

    # ---- DRAM I/O ----
    gdt = wdt if GATHER_BF16 else f32
    emb_dram = nc.dram_tensor("emb", [VOCAB, EMBED], gdt,
                              kind="ExternalInput")
    if WIDE_GATHER:
        idxw_dram = nc.dram_tensor("idxw", [N_ITERS * CHAINS, 128, TPC],
                                   i32, kind="ExternalInput")
    # 24 stationary tiles per gate-chunk m: (m, k<2) = Wh block, (m, 2) = Wx
    whx_dram = nc.dram_tensor("whxT", [128, 24 * 128], wdt,
                              kind="ExternalInput")
    bb_dram = nc.dram_tensor("bbT", [128, GB], wdt, kind="ExternalInput")
    wdT_dram = nc.dram_tensor("wdT", [128, 8], f32, kind="ExternalInput")
    idf_dram = nc.dram_tensor("identf", [128, 128], f32, kind="ExternalInput")
    idw_dram = nc.dram_tensor("identw", [128, 128], wdt, kind="ExternalInput")
    idx_dram = nc.dram_tensor("idx", [128, N_ITERS * CHAINS * TPC], i32,
                              kind="ExternalInput")
    out_dram = nc.dram_tensor("out", [CHAINS, NUM_CLASSES, B], f32,
                              kind="ExternalOutput")

    with TileContext(nc) as tc:
        with (
            tc.tile_pool(name="const", bufs=1) as constp,
            tc.tile_pool(name="state", bufs=1) as statep,

            tc.tile_pool(name="embp", bufs=8) as embp,
            tc.tile_pool(name="idxwp", bufs=3) as idxwp,
            tc.tile_pool(name="embTp", bufs=3) as embTp,
            tc.tile_pool(name="sgp", bufs=2) as sgp,
            tc.tile_pool(name="sop", bufs=2) as sop,
            tc.tile_pool(name="tmpp", bufs=2) as tmpp,
            tc.tile_pool(name="outp", bufs=1) as outp,
            tc.tile_pool(name="zps0", bufs=(1 if SEQV == "quad" else 2),
                         space="PSUM") as zps0,
            tc.tile_pool(name="zps1", bufs=(1 if SEQV == "quad" else 2),
                         space="PSUM") as zps1,
            tc.tile_pool(name="trps", bufs=2, space="PSUM") as trps,
            tc.tile_pool(name="dps", bufs=1, space="PSUM") as dps,
        ):
            zps = [zps0, zps1]

            # ---- load constants ----
            whx = constp.tile([128, 24 * 128], wdt)
            bb = constp.tile([128, GB], wdt)
            wdT = constp.tile([128, 8], f32)
            idf = constp.tile([128, 128], f32)
            idw = constp.tile([128, 128], wdt)
            idx_sb = constp.tile([128, N_ITERS * CHAINS * TPC], i32,
                                 name="idx_sb")
            # idx gates the first gather and idf the first transpose; load
            # them before the large whx tensor so the prologue overlaps.
            for dst, src in ((idx_sb, idx_dram), (idf, idf_dram)):
                nc.sync.dma_start(out=dst[:], in_=src[:])

            def emit_const_dmas():
                for dst, src in ((whx, whx_dram), (bb, bb_dram),
                                 (wdT, wdT_dram), (idw, idw_dram)):
                    nc.sync.dma_start(out=dst[:], in_=src[:])

            # ---- per-chain persistent state ----
            hT = [statep.tile([128, 2 * B], wdt, tag=f"hT{c}",
                              name=f"hT{c}") for c in range(CHAINS)]
            cst = [statep.tile([128, 2 * B], f32, tag=f"c{c}",
                               name=f"cst{c}") for c in range(CHAINS)]
            for c in range(CHAINS):
                nc.vector.memset(hT[c][:], 0.0)
                nc.vector.memset(cst[c][:], 0.0)

            def emit_precompute(it):
                """Gather + transpose emb block for iteration `it`.
                Returns (dma_units, xf_units, embT tiles): DMA launches are
                emitted early in the iteration; the PE transposes + copies
                are deferred until the gathers are surely complete so they
                never head-of-line-block the recurrence matmuls."""
                dma_units, xf_units = [], []
                # one tile per (chain, slice): WAR on an overwrite releases
                # as soon as that slice's last x-projection read retires,
                # instead of waiting for the whole iteration's reads.
                embTs = [[embTp.tile([128, 128], wdt, tag=f"embT{c}{j}",
                                     name=f"embT{c}{j}")
                          for j in range(TPC)] for c in range(CHAINS)]
                ets = {}
                base = it * CHAINS * TPC
                if WIDE_GATHER:
                    for c in range(CHAINS):
                        def g_unit(c=c):
                            # ucode requires the index tile to be a packed,
                            # directly-DMA'd [128, TPC] tensor: load this
                            # (iter, chain)'s slab from DRAM, then gather
                            # all TPC slices in one SWDGE instruction.
                            idxw = idxwp.tile([128, TPC], i32,
                                              tag=f"idxw{c}", name=f"idxw{c}")
                            nc.sync.dma_start(
                                out=idxw[:],
                                in_=idxw_dram[it * CHAINS + c])
                            et = embp.tile([128, TPC * 128], gdt,
                                           tag=f"emb{c}", name=f"emb{c}")
                            for j in range(TPC):
                                ets[(c, j)] = et[:, j * 128:(j + 1) * 128]
                            nc.gpsimd.indirect_dma_start(
                                out=et[:], out_offset=None, in_=emb_dram[:],
                                in_offset=bass.IndirectOffsetOnAxis(
                                    ap=idxw[:], axis=0))
                        dma_units.append(g_unit)
                    for j in range(TPC):
                        for c in range(CHAINS):
                            def x_unit(c=c, j=j):
                                tp = trps.tile([128, 128], gdt, name="tp")
                                nc.tensor.transpose(
                                    out=tp[:], in_=ets[(c, j)],
                                    identity=(idw[:] if GATHER_BF16
                                              else idf[:]))
                                nc.vector.tensor_copy(
                                    out=embTs[c][j][:], in_=tp[:])
                            xf_units.append(x_unit)
                    return dma_units, xf_units, embTs
                for j in range(TPC):
                    for c in range(CHAINS):
                        def g_unit(c=c, j=j):
                            et = embp.tile([128, 128], gdt, tag=f"emb{c}{j}",
                                           name=f"emb{c}{j}")
                            ets[(c, j)] = et
                            nc.gpsimd.indirect_dma_start(
                                out=et[:], out_offset=None, in_=emb_dram[:],
                                in_offset=bass.IndirectOffsetOnAxis(
                                    ap=idx_sb[:, base + c * TPC + j:
                                              base + c * TPC + j + 1],
                                    axis=0))
                        def x_unit(c=c, j=j):
                            tp = trps.tile([128, 128], gdt, name="tp")
                            nc.tensor.transpose(
                                out=tp[:], in_=ets[(c, j)][:],
                                identity=(idw[:] if GATHER_BF16
                                          else idf[:]))
                            nc.vector.tensor_copy(
                                out=embTs[c][j][:], in_=tp[:])
                        dma_units.append(g_unit)
                        xf_units.append(x_unit)
                return dma_units, xf_units, embTs

            # ---- pipeline state ----
            ztile = [None] * CHAINS        # PSUM z for the in-flight step
            sgt = [None] * CHAINS
            sot = [None] * CHAINS
            tct = [None] * CHAINS

            def zsl(c, m):
                """column slice of z for gate-block m (0..7)."""
                return ztile[c][:, m * B:(m + 1) * B]

            def emit_x(c, s, embT_c):
                """Create step-s PSUM tile; bias + 8 emb-projection matmuls."""
                ztile[c] = zps[c % 2].tile([128, 8 * B], f32, tag=f"z{c}",
                                           name=f"z{c}")
                if with_bias:
                    nc.tensor.matmul(
                        out=ztile[c][:], lhsT=idw[:], rhs=bb[:],
                        start=True, stop=False, skip_group_check=True)
                sl = s % STEPS
                j, jo = sl * B // 128, (sl * B) % 128
                emb_s = embT_c[j][:, jo:jo + B]
                for m in range(8):
                    nc.tensor.matmul(
                        out=zsl(c, m),
                        lhsT=whx[:, (m * 3 + 2) * 128:(m * 3 + 3) * 128],
                        rhs=emb_s,
                        start=(not with_bias and m == 0),
                        stop=False, skip_group_check=True)

            def emit_ifg_mms(c, hi=6):
                for k in range(2):
                    for m in range(hi):
                        nc.tensor.matmul(
                            out=zsl(c, m),
                            lhsT=whx[:, (m * 3 + k) * 128:
                                     (m * 3 + k + 1) * 128],
                            rhs=hT[c][:, k * B:(k + 1) * B],
                            start=False, stop=(k == 1 and m == hi - 1),
                            skip_group_check=True)

            def emit_o_mms(c):
                for k in range(2):
                    for m in range(6, 8):
                        nc.tensor.matmul(
                            out=zsl(c, m),
                            lhsT=whx[:, (m * 3 + k) * 128:
                                     (m * 3 + k + 1) * 128],
                            rhs=hT[c][:, k * B:(k + 1) * B],
                            start=False, stop=(k == 1 and m == 7),
                            skip_group_check=True)

            def emit_sig(c, hi=6):
                sg = sgp.tile([128, hi * B], f32, tag=f"sg{c}",
                              name=f"sg{c}")
                sgt[c] = sg
                nc.scalar.activation(out=sg[:], in_=ztile[c][:, 0:hi * B],
                                     func=SIG)

            def emit_sigo(c):
                so = sop.tile([128, 2 * B], wdt, tag=f"so{c}", name=f"so{c}")
                sot[c] = so
                nc.scalar.activation(out=so[:], in_=ztile[c][:, 6 * B:8 * B],
                                     func=SIG)

            T1_POOL = os.environ.get("KNOB_T1_POOL", "0") == "1"

            def emit_cupd(c):
                sg = sgt[c]
                t2 = tmpp.tile([128, 2 * B], f32, tag=f"t2{c}", name=f"t2{c}")
                t1 = tmpp.tile([128, 2 * B], f32, tag=f"t1{c}", name=f"t1{c}")
                # t2 = (sig_g-0.5)*sig_i ; t1 = sig_f*c ; c = 2*t2 + t1
                nc.vector.scalar_tensor_tensor(
                    out=t2[:], in0=sg[:, 4 * B:6 * B], scalar=0.5,
                    in1=sg[:, 0:2 * B], op0=SUB, op1=MULT)
                if T1_POOL:
                    nc.gpsimd.tensor_mul(
                        out=t1[:], in0=sg[:, 2 * B:4 * B], in1=cst[c][:])
                else:
                    nc.vector.tensor_mul(
                        out=t1[:], in0=sg[:, 2 * B:4 * B], in1=cst[c][:])
                nc.vector.scalar_tensor_tensor(
                    out=cst[c][:], in0=t2[:], scalar=2.0,
                    in1=t1[:], op0=MULT, op1=ADD)

            def emit_tanh(c):
                tcc = tmpp.tile([128, 2 * B], wdt, tag=f"tc{c}",
                                name=f"tc{c}")
                tct[c] = tcc
                nc.scalar.activation(out=tcc[:], in_=cst[c][:], func=TANH)

            def emit_h(c):
                # h = sigmoid(z_o) * tanh(c)   (all-bf16 tensor-tensor, 2x)
                so = sot[c][:] if sot[c] is not None else \
                    sgt[c][:, 6 * B:8 * B]
                nc.vector.tensor_mul(out=hT[c][:], in0=so,
                                     in1=tct[c][:])

            # ---- prologue: first gathers + step-0 x-projections ----
            # launch the j<=1 gathers, transpose j0, start step 0; the
            # rest of iteration 0's and all of iteration 1's prefetch
            # drains inside the first rounds (the loop prefetches two
            # iterations ahead from then on).
            dma_u, xf_u, embT = emit_precompute(0)
            for u in dma_u[:2 * CHAINS]:
                u()
            # large constant loads after the first gathers so their DMA
            # transfers don't queue behind the 2.2us whx transfer
            emit_const_dmas()
            # warm the PE p-state ramp with dummy transposes (the ramp
            # needs ~3us of activity before matmuls hit full clock)
            for _ in range(14):
                tpd = trps.tile([128, 128], gdt, name="tp")
                nc.tensor.transpose(out=tpd[:], in_=idf[:], identity=idf[:])
            for u in xf_u[:CHAINS]:
                u()
            for c in range(CHAINS):
                emit_x(c, 0, embT[c])
            for u in xf_u[CHAINS:2 * CHAINS]:
                u()
            dma_u = dma_u[2 * CHAINS:]
            xf_u = xf_u[2 * CHAINS:]
            dma_u1, xf_u1, embT_nxt = emit_precompute(1)
            dma_u += dma_u1
            xf_u += xf_u1
            dma_nx, xf_nx = [], []
            embT_nx2 = None

            A, Bc = 0, 1
            for r in range(T_FULL):
                it = r // STEPS
                if r % STEPS == 0 and it + 2 < N_ITERS:
                    dma_nx, xf_nx, embT_nx2 = emit_precompute(it + 2)
                if (r + 1) % STEPS == 0:
                    # next round's x-projections read embT_nxt: flush all
                    # remaining gather/transpose writes before those reads.
                    while dma_u:
                        dma_u.pop(0)()
                    while xf_u:
                        xf_u.pop(0)()
                    dma_u, dma_nx = dma_nx, []
                    xf_u, xf_nx = xf_nx, []

                def x_next(c):
                    if r + 1 >= T_FULL:
                        return
                    src = embT_nxt if (r + 1) % STEPS == 0 else embT
                    emit_x(c, r + 1, src[c])

                if SEQV == "quad":
                    # 4 chains at quarter-round offsets, merged sigma(ifgo)
                    for c, ph in ((0, 1), (3, 2), (1, 1), (0, 2),
                                  (2, 1), (1, 2), (3, 1), (2, 2)):
                        if ph == 1:
                            sot[c] = None
                            emit_ifg_mms(c, hi=8)
                            emit_sig(c, hi=8)
                            x_next(c)
                        else:
                            if r == 0 and c == 3:
                                continue
                            emit_cupd(c); emit_tanh(c); emit_h(c)
                elif SEQV == "lockstep":
                    emit_ifg_mms(A); emit_sig(A)
                    emit_ifg_mms(Bc); emit_sig(Bc)
                    emit_o_mms(A); emit_sigo(A); x_next(A)
                    emit_o_mms(Bc); emit_sigo(Bc); x_next(Bc)
                    emit_cupd(A); emit_tanh(A); emit_h(A)
                    emit_cupd(Bc); emit_tanh(Bc); emit_h(Bc)
                else:
                    # half-step offset: A runs step r in the first half of
                    # the round, B's step r-1 back-half completes, then B
                    # starts step r in the second half. The last step only
                    # needs c (the model output), so its o/tanh/h are dead.
                    last = r == T_FULL - 1
                    emit_ifg_mms(A); emit_sig(A)
                    if r > 0:
                        emit_cupd(Bc)          # B's step r-1 c-chain
                        emit_tanh(Bc)
                    if not last:
                        emit_o_mms(A); emit_sigo(A)
                    x_next(A)
                    if r > 0:
                        emit_h(Bc)
                    emit_cupd(A)
                    if not last:
                        emit_tanh(A)
                    emit_ifg_mms(Bc); emit_sig(Bc)
                    if not last:
                        emit_h(A)
                        emit_o_mms(Bc); emit_sigo(Bc)
                    x_next(Bc)

                # spread next iteration's prefetch: DMA launches first
                # (2/round), deferred transposes+copies later (1/round).
                for _ in range(2):
                    if dma_u:
                        dma_u.pop(0)()
                if xf_u and (r >= 4 if it == 0 else
                             (not dma_u and r % STEPS >= 4)):
                    xf_u.pop(0)()
                if r % STEPS == STEPS - 1 and embT_nxt is not None:
                    embT = embT_nxt
                    embT_nxt = embT_nx2
                    embT_nx2 = None

            def emit_epilogue(c):
                # dense epilogue: partial logits = (Wd_half)^T @ c
                dp = dps.tile([NUM_CLASSES, B], f32, tag=f"dp{c}",
                              name=f"dp{c}")
                for k in range(2):
                    nc.tensor.matmul(
                        out=dp[:], lhsT=wdT[:, k * 4:(k + 1) * 4],
                        rhs=cst[c][:, k * B:(k + 1) * B],
                        start=(k == 0), stop=(k == 1))
                ob = outp.tile([NUM_CLASSES, B], f32, tag=f"ob{c}",
                               name=f"ob{c}")
                nc.vector.tensor_copy(out=ob[:], in_=dp[:])
                nc.sync.dma_start(out=out_dram[c], in_=ob[:])

            if SEQV == "quad":
                emit_cupd(3)
                for c in range(CHAINS):
                    emit_epilogue(c)
            elif SEQV != "lockstep":
                emit_epilogue(A)
                emit_cupd(Bc)
                emit_epilogue(Bc)
            else:
                for c in range(CHAINS):
                    emit_epilogue(c)

    nc.compile()
    return nc


def _prep_core_inputs(core, x, emb_np, Wx, Wh, b, Wd):
    """Host-side prep: weight layout/scaling + gather index schedule."""
    d, s = core // 4, core % 4
    Wx = Wx.astype(np.float32).copy()
    Wh = Wh.astype(np.float32).copy()
    b = b.astype(np.float32).copy()
    # fold tanh(z_g) = 2*sigmoid(2 z_g) - 1: double the g-gate inputs.
    Wx[:, 512:768] *= 2.0
    b[512:768] *= 2.0
    Wh[:, 512:768] *= 2.0

    whx = np.empty((128, 24 * 128), np.float32)
    for m in range(8):
        for k in range(2):
            whx[:, (m * 3 + k) * 128:(m * 3 + k + 1) * 128] = \
                Wh[k * 128:(k + 1) * 128, m * 128:(m + 1) * 128]
        whx[:, (m * 3 + 2) * 128:(m * 3 + 3) * 128] = \
            Wx[:, m * 128:(m + 1) * 128]
    bb = np.repeat(b.reshape(8, 128).T[:, :, None], B, axis=2).reshape(128, GB)
    wdT = np.empty((128, 8), np.float32)
    for k in range(2):
        wdT[:, k * 4:(k + 1) * 4] = Wd[d * 256 + k * 128:
                                       d * 256 + (k + 1) * 128, :]

    it = np.arange(N_ITERS)[:, None, None]
    p = np.arange(128)[None, :, None]
    cj = np.arange(CHAINS * TPC)[None, None, :]
    chain, j = cj // TPC, cj % TPC
    s_local = j * (128 // B) + p // B
    jb = p % B
    t = it * STEPS + s_local
    if d == 1:
        t = (T_FULL - 1) - t
    row = s * 64 + chain * B + jb
    idx = x[row, t].astype(np.int32)          # [N_ITERS, 128, CHAINS*TPC]
    idxw = np.ascontiguousarray(
        idx.reshape(N_ITERS, 128, CHAINS, TPC).transpose(0, 2, 1, 3)
           .reshape(N_ITERS * CHAINS, 128, TPC))
    idx = np.ascontiguousarray(idx.transpose(1, 0, 2).reshape(128, -1))

    ins = {
        "emb": emb_np,
        "whxT": np.ascontiguousarray(whx.astype(W_NP)),
        "bbT": np.ascontiguousarray(bb.astype(W_NP)),
        "wdT": wdT,
        "identf": np.eye(128, dtype=np.float32),
        "identw": np.eye(128).astype(W_NP),
        "idx": idx,
    }
    if WIDE_GATHER:
        ins["idxw"] = idxw
    return ins


def kernel(x, train, embed_table, Wx_f, Wh_f, b_f, Wx_b, Wh_b, b_b, Wd, bd,
           **_unused):
    from concourse.bass_utils import run_bass_kernel_spmd

    x = np.asarray(x).astype(np.int64)
    emb_np = np.ascontiguousarray(
        np.asarray(embed_table, np.float32).astype(G_NP))
    Wd_np = np.asarray(Wd, np.float32)

    with_bias = bool(np.any(np.asarray(b_f)) or np.any(np.asarray(b_b)))
    key = ("nc", with_bias)
    if key not in _CACHE:
        _CACHE[key] = _build_program(with_bias)
    nc = _CACHE[key]

    in_maps = []
    for core in range(N_CORES):
        if core < 4:
            Wx, Wh, b = Wx_f, Wh_f, b_f
        else:
            Wx, Wh, b = Wx_b, Wh_b, b_b
        in_maps.append(_prep_core_inputs(
            core, x, emb_np, np.asarray(Wx), np.asarray(Wh), np.asarray(b),
            Wd_np))

    res = run_bass_kernel_spmd(nc, in_maps, list(range(N_CORES))).results

    logits = np.zeros((B_FULL, NUM_CLASSES), np.float32)
    for core in range(N_CORES):
        s = core % 4
        o = np.asarray(res[core]["out"], np.float32)  # [CHAINS, 4, B]
        for c in range(CHAINS):
            r0 = s * 64 + c * B
            logits[r0:r0 + B] += o[c].T
    logits += np.asarray(bd, np.float32)[None, :]
    return logits


# revision 76
# speedup vs baseline: 1.0008x; 1.0005x over previous
"""BiLSTM classifier Trainium2 kernel (8 NeuronCores, SPMD).

Model (reference): emb = table[x]; c_f = LSTM_final_cell(emb, fwd);
c_b = LSTM_final_cell(flip(emb), bwd); out = [c_f, c_b] @ Wd + bd.

Sharding: 8 cores = 2 directions x 4 batch-shards of 64 rows; each core runs
2 independent LSTM "chains" of batch 32, software-pipelined HALF A STEP apart
so each chain's serial step latency (the wall-clock limiter: PE -> sigmoid ->
c-update -> tanh -> h-update, ~2.17us/step in the timeline model) overlaps
the other chain's engine time instead of serializing with it. All state is
TRANSPOSED on-chip: gates/hidden on partitions, batch along the free dim.

Per step (per chain), z^T accumulates in one PSUM bank laid out
[i i f f g g o o] (B cols per 128-gate block):
  z^T = Wx[m]^T @ emb_t^T   (8 matmuls, no h dependency -> issued one round
                             early, during the previous step)
      + Wh[k,m]^T @ h^T[k]  (16 matmuls; only the 12 i,f,g ones gate the
                             critical path; o's run after)
then
  sg = sigmoid(z_ifg)            (one Act op; g pre-doubled via host fold so
                                  sg_g = (tanh(z_g)+1)/2)
  so = sigmoid(z_o)              (Act, off critical path, bf16)
  t2 = (sg_g-0.5)*sg_i ; t1 = sg_f*c ; c = 2*t2 + t1   (DVE stt/tt/stt)
  tc = tanh(c)                   (Act, bf16)
  h  = so * tc                   (DVE tensor-tensor, all-bf16 2x mode)
The last step emits only the c-path (o/tanh/h are dead there).

emb^T comes from an indirect-DMA gather of embedding rows (128 tokens/instr,
schedule precomputed on host; the full index table is preloaded once) + PE
transpose + copy. Gather DMAs are launched early in the PRIOR iteration and
the transposes/copies deferred until the data is long since landed, with one
embT tile per (chain, 128-token slice) so overwrite WARs release per-slice
and never collide with the iteration boundary. Final: partial logits
(4 x 32) = Wd_half^T @ c per chain, summed across direction pairs on host.
"""

import sys

for _p in ("/root/.axon_site/_ro/trn_rl_repo", "/opt/trn_rl_repo"):
    if _p not in sys.path:
        sys.path.insert(0, _p)

import numpy as np
import ml_dtypes

# ---- problem constants (hardcoded; kernel.py must be self-contained) ----
VOCAB = 32000
EMBED = 128
HIDDEN = 256
NUM_CLASSES = 4
B_FULL, T_FULL = 256, 512

import os
N_CORES = 8
CHAINS = int(os.environ.get("KNOB_CHAINS", "2"))
B = 64 // CHAINS    # batch per chain
STEPS = 16          # time steps per iteration block
N_ITERS = T_FULL // STEPS
GB = 8 * B          # gate-row block per step in z^T layout ( = 4H/128 * B )
TPC = STEPS * B // 128      # gather tiles per chain per iteration
W_NP = ml_dtypes.bfloat16   # on-chip matmul operand dtype
SEQV = os.environ.get("KNOB_SEQ", "offset")   # emission-order variant
WIDE_GATHER = os.environ.get("KNOB_WIDE_GATHER", "0") == "1"
# multi-index gathers (WIDE) and bf16 gathers mis-route data on HW when
# combined with the rest of the pipeline; keep validated f32 single-index
# gathers by default.
GATHER_BF16 = (os.environ.get("KNOB_GATHER_BF16", "0") == "1"
               and not WIDE_GATHER)
G_NP = ml_dtypes.bfloat16 if GATHER_BF16 else np.float32

_CACHE = {}


def _build_program(with_bias=True):
    import concourse.bacc as bacc
    import concourse.mybir as mybir
    from concourse import bass
    from concourse.tile import TileContext

    f32 = mybir.dt.float32
    i32 = mybir.dt.int32
    wdt = mybir.dt.bfloat16
    SIG = mybir.ActivationFunctionType.Sigmoid
    TANH = mybir.ActivationFunctionType.Tanh
    MULT = mybir.AluOpType.mult
    ADD = mybir.AluOpType.add
    SUB = mybir.AluOpType.subtract

    nc = bacc.Bacc("TRN2", target_bir_lowering=False, debug=False,
                   num_devices=N_CORES,
                   dynamic_dma_scratch_size=int(os.environ.get(
                       "KNOB_DMA_SCRATCH", "16384")))

    # ---- DRAM I/O ----
    gdt = wdt if GATHER_BF16 else f32
    emb_dram = nc.dram_tensor("emb", [VOCAB, EMBED], gdt,
                              kind="ExternalInput")
    if WIDE_GATHER:
        idxw_dram = nc.dram_tensor("idxw", [N_ITERS * CHAINS, 128, TPC],
                                   i32, kind="ExternalInput")
    # 24 stationary tiles per gate-chunk m: (m, k<2) = Wh block, (m, 2) = Wx
    whx_dram = nc.dram_tensor("whxT", [128, 24 * 128], wdt,
                              kind="ExternalInput")
    bb_dram = nc.dram_tensor("bbT", [128, GB], wdt, kind="ExternalInput")
    wdT_dram = nc.dram_tensor("wdT", [128, 8], f32, kind="ExternalInput")
    idf_dram = nc.dram_tensor("identf", [128, 128], f32, kind="ExternalInput")
    idw_dram = nc.dram_tensor("identw", [128, 128], wdt, kind="ExternalInput")
    idx_dram = nc.dram_tensor("idx", [128, N_ITERS * CHAINS * TPC], i32,
                              kind="ExternalInput")
    out_dram = nc.dram_tensor("out", [CHAINS, NUM_CLASSES, B], f32,
                              kind="ExternalOutput")

    with TileContext(nc) as tc:
        with (
            tc.tile_pool(name="const", bufs=1) as constp,
            tc.tile_pool(name="state", bufs=1) as statep,

            tc.tile_pool(name="embp", bufs=8) as embp,
            tc.tile_pool(name="idxwp", bufs=3) as idxwp,
            tc.tile_pool(name="embTp", bufs=3) as embTp,
            tc.tile_pool(name="sgp", bufs=2) as sgp,
            tc.tile_pool(name="sop", bufs=2) as sop,
            tc.tile_pool(name="tmpp", bufs=2) as tmpp,
            tc.tile_pool(name="outp", bufs=1) as outp,
            tc.tile_pool(name="zps0", bufs=(1 if SEQV == "quad" else 2),
                         space="PSUM") as zps0,
            tc.tile_pool(name="zps1", bufs=(1 if SEQV == "quad" else 2),
                         space="PSUM") as zps1,
            tc.tile_pool(name="trps", bufs=2, space="PSUM") as trps,
            tc.tile_pool(name="dps", bufs=1, space="PSUM") as dps,
        ):
            zps = [zps0, zps1]

            # ---- load constants ----
            whx = constp.tile([128, 24 * 128], wdt)
            bb = constp.tile([128, GB], wdt)
            wdT = constp.tile([128, 8], f32)
            idf = constp.tile([128, 128], f32)
            idw = constp.tile([128, 128], wdt)
            idx_sb = constp.tile([128, N_ITERS * CHAINS * TPC], i32,
                                 name="idx_sb")
            # idx gates the first gather and idf the first transpose; load
            # them before the large whx tensor so the prologue overlaps.
            # The first two iterations' index columns go in a small first
            # DMA so gather descgen isn't gated on the full table transfer.
            ncol0 = 2 * CHAINS * TPC
            nc.sync.dma_start(out=idx_sb[:, 0:ncol0],
                              in_=idx_dram[:, 0:ncol0])
            nc.sync.dma_start(out=idf[:], in_=idf_dram[:])
            nc.sync.dma_start(out=idx_sb[:, ncol0:],
                              in_=idx_dram[:, ncol0:])

            def emit_const_dmas():
                for dst, src in ((whx, whx_dram), (bb, bb_dram),
                                 (wdT, wdT_dram), (idw, idw_dram)):
                    nc.sync.dma_start(out=dst[:], in_=src[:])

            # ---- per-chain persistent state ----
            hT = [statep.tile([128, 2 * B], wdt, tag=f"hT{c}",
                              name=f"hT{c}") for c in range(CHAINS)]
            cst = [statep.tile([128, 2 * B], f32, tag=f"c{c}",
                               name=f"cst{c}") for c in range(CHAINS)]
            for c in range(CHAINS):
                nc.vector.memset(hT[c][:], 0.0)
                nc.vector.memset(cst[c][:], 0.0)

            def emit_precompute(it):
                """Gather + transpose emb block for iteration `it`.
                Returns (dma_units, xf_units, embT tiles): DMA launches are
                emitted early in the iteration; the PE transposes + copies
                are deferred until the gathers are surely complete so they
                never head-of-line-block the recurrence matmuls."""
                dma_units, xf_units = [], []
                # one tile per (chain, slice): WAR on an overwrite releases
                # as soon as that slice's last x-projection read retires,
                # instead of waiting for the whole iteration's reads.
                embTs = [[embTp.tile([128, 128], wdt, tag=f"embT{c}{j}",
                                     name=f"embT{c}{j}")
                          for j in range(TPC)] for c in range(CHAINS)]
                ets = {}
                base = it * CHAINS * TPC
                if WIDE_GATHER:
                    for c in range(CHAINS):
                        def g_unit(c=c):
                            # ucode requires the index tile to be a packed,
                            # directly-DMA'd [128, TPC] tensor: load this
                            # (iter, chain)'s slab from DRAM, then gather
                            # all TPC slices in one SWDGE instruction.
                            idxw = idxwp.tile([128, TPC], i32,
                                              tag=f"idxw{c}", name=f"idxw{c}")
                            nc.sync.dma_start(
                                out=idxw[:],
                                in_=idxw_dram[it * CHAINS + c])
                            et = embp.tile([128, TPC * 128], gdt,
                                           tag=f"emb{c}", name=f"emb{c}")
                            for j in range(TPC):
                                ets[(c, j)] = et[:, j * 128:(j + 1) * 128]
                            nc.gpsimd.indirect_dma_start(
                                out=et[:], out_offset=None, in_=emb_dram[:],
                                in_offset=bass.IndirectOffsetOnAxis(
                                    ap=idxw[:], axis=0))
                        dma_units.append(g_unit)
                    for j in range(TPC):
                        for c in range(CHAINS):
                            def x_unit(c=c, j=j):
                                tp = trps.tile([128, 128], gdt, name="tp")
                                nc.tensor.transpose(
                                    out=tp[:], in_=ets[(c, j)],
                                    identity=(idw[:] if GATHER_BF16
                                              else idf[:]))
                                nc.vector.tensor_copy(
                                    out=embTs[c][j][:], in_=tp[:])
                            xf_units.append(x_unit)
                    return dma_units, xf_units, embTs
                for j in range(TPC):
                    for c in range(CHAINS):
                        def g_unit(c=c, j=j):
                            et = embp.tile([128, 128], gdt, tag=f"emb{c}{j}",
                                           name=f"emb{c}{j}")
                            ets[(c, j)] = et
                            nc.gpsimd.indirect_dma_start(
                                out=et[:], out_offset=None, in_=emb_dram[:],
                                in_offset=bass.IndirectOffsetOnAxis(
                                    ap=idx_sb[:, base + c * TPC + j:
                                              base + c * TPC + j + 1],
                                    axis=0))
                        def x_unit(c=c, j=j):
                            tp = trps.tile([128, 128], gdt, name="tp")
                            nc.tensor.transpose(
                                out=tp[:], in_=ets[(c, j)][:],
                                identity=(idw[:] if GATHER_BF16
                                          else idf[:]))
                            nc.vector.tensor_copy(
                                out=embTs[c][j][:], in_=tp[:])
                        dma_units.append(g_unit)
                        xf_units.append(x_unit)
                return dma_units, xf_units, embTs

            # ---- pipeline state ----
            ztile = [None] * CHAINS        # PSUM z for the in-flight step
            sgt = [None] * CHAINS
            sot = [None] * CHAINS
            tct = [None] * CHAINS

            def zsl(c, m):
                """column slice of z for gate-block m (0..7)."""
                return ztile[c][:, m * B:(m + 1) * B]

            def emit_x(c, s, embT_c):
                """Create step-s PSUM tile; bias + 8 emb-projection matmuls."""
                ztile[c] = zps[c % 2].tile([128, 8 * B], f32, tag=f"z{c}",
                                           name=f"z{c}")
                if with_bias:
                    nc.tensor.matmul(
                        out=ztile[c][:], lhsT=idw[:], rhs=bb[:],
                        start=True, stop=False, skip_group_check=True)
                sl = s % STEPS
                j, jo = sl * B // 128, (sl * B) % 128
                emb_s = embT_c[j][:, jo:jo + B]
                for m in range(8):
                    nc.tensor.matmul(
                        out=zsl(c, m),
                        lhsT=whx[:, (m * 3 + 2) * 128:(m * 3 + 3) * 128],
                        rhs=emb_s,
                        start=(not with_bias and m == 0),
                        stop=False, skip_group_check=True)

            def emit_ifg_mms(c, hi=6):
                for k in range(2):
                    for m in range(hi):
                        nc.tensor.matmul(
                            out=zsl(c, m),
                            lhsT=whx[:, (m * 3 + k) * 128:
                                     (m * 3 + k + 1) * 128],
                            rhs=hT[c][:, k * B:(k + 1) * B],
                            start=False, stop=(k == 1 and m == hi - 1),
                            skip_group_check=True)

            def emit_o_mms(c):
                for k in range(2):
                    for m in range(6, 8):
                        nc.tensor.matmul(
                            out=zsl(c, m),
                            lhsT=whx[:, (m * 3 + k) * 128:
                                     (m * 3 + k + 1) * 128],
                            rhs=hT[c][:, k * B:(k + 1) * B],
                            start=False, stop=(k == 1 and m == 7),
                            skip_group_check=True)

            def emit_sig(c, hi=6):
                sg = sgp.tile([128, hi * B], f32, tag=f"sg{c}",
                              name=f"sg{c}")
                sgt[c] = sg
                nc.scalar.activation(out=sg[:], in_=ztile[c][:, 0:hi * B],
                                     func=SIG)

            def emit_sigo(c):
                so = sop.tile([128, 2 * B], wdt, tag=f"so{c}", name=f"so{c}")
                sot[c] = so
                nc.scalar.activation(out=so[:], in_=ztile[c][:, 6 * B:8 * B],
                                     func=SIG)

            T1_POOL = os.environ.get("KNOB_T1_POOL", "0") == "1"

            def emit_cupd(c):
                sg = sgt[c]
                t2 = tmpp.tile([128, 2 * B], f32, tag=f"t2{c}", name=f"t2{c}")
                t1 = tmpp.tile([128, 2 * B], f32, tag=f"t1{c}", name=f"t1{c}")
                # t2 = (sig_g-0.5)*sig_i ; t1 = sig_f*c ; c = 2*t2 + t1
                nc.vector.scalar_tensor_tensor(
                    out=t2[:], in0=sg[:, 4 * B:6 * B], scalar=0.5,
                    in1=sg[:, 0:2 * B], op0=SUB, op1=MULT)
                if T1_POOL:
                    nc.gpsimd.tensor_mul(
                        out=t1[:], in0=sg[:, 2 * B:4 * B], in1=cst[c][:])
                else:
                    nc.vector.tensor_mul(
                        out=t1[:], in0=sg[:, 2 * B:4 * B], in1=cst[c][:])
                nc.vector.scalar_tensor_tensor(
                    out=cst[c][:], in0=t2[:], scalar=2.0,
                    in1=t1[:], op0=MULT, op1=ADD)

            def emit_tanh(c):
                tcc = tmpp.tile([128, 2 * B], wdt, tag=f"tc{c}",
                                name=f"tc{c}")
                tct[c] = tcc
                nc.scalar.activation(out=tcc[:], in_=cst[c][:], func=TANH)

            def emit_h(c):
                # h = sigmoid(z_o) * tanh(c)   (all-bf16 tensor-tensor, 2x)
                so = sot[c][:] if sot[c] is not None else \
                    sgt[c][:, 6 * B:8 * B]
                nc.vector.tensor_mul(out=hT[c][:], in0=so,
                                     in1=tct[c][:])

            # ---- prologue: first gathers + step-0 x-projections ----
            # launch the j<=1 gathers, transpose j0, start step 0; the
            # rest of iteration 0's and all of iteration 1's prefetch
            # drains inside the first rounds (the loop prefetches two
            # iterations ahead from then on).
            dma_u, xf_u, embT = emit_precompute(0)
            for u in dma_u[:2 * CHAINS]:
                u()
            # large constant loads after the first gathers so their DMA
            # transfers don't queue behind the 2.2us whx transfer
            emit_const_dmas()
            # warm the PE p-state ramp with dummy transposes (the ramp
            # needs ~3us of activity before matmuls hit full clock)
            for _ in range(14):
                tpd = trps.tile([128, 128], gdt, name="tp")
                nc.tensor.transpose(out=tpd[:], in_=idf[:], identity=idf[:])
            for u in xf_u[:CHAINS]:
                u()
            for c in range(CHAINS):
                emit_x(c, 0, embT[c])
            for u in xf_u[CHAINS:2 * CHAINS]:
                u()
            dma_u = dma_u[2 * CHAINS:]
            xf_u = xf_u[2 * CHAINS:]
            dma_u1, xf_u1, embT_nxt = emit_precompute(1)
            dma_u += dma_u1
            xf_u += xf_u1
            dma_nx, xf_nx = [], []
            embT_nx2 = None

            A, Bc = 0, 1
            for r in range(T_FULL):
                it = r // STEPS
                if r % STEPS == 0 and it + 2 < N_ITERS:
                    dma_nx, xf_nx, embT_nx2 = emit_precompute(it + 2)
                if (r + 1) % STEPS == 0:
                    # next round's x-projections read embT_nxt: flush all
                    # remaining gather/transpose writes before those reads.
                    while dma_u:
                        dma_u.pop(0)()
                    while xf_u:
                        xf_u.pop(0)()
                    dma_u, dma_nx = dma_nx, []
                    xf_u, xf_nx = xf_nx, []

                def x_next(c):
                    if r + 1 >= T_FULL:
                        return
                    src = embT_nxt if (r + 1) % STEPS == 0 else embT
                    emit_x(c, r + 1, src[c])

                if SEQV == "quad":
                    # 4 chains at quarter-round offsets, merged sigma(ifgo)
                    for c, ph in ((0, 1), (3, 2), (1, 1), (0, 2),
                                  (2, 1), (1, 2), (3, 1), (2, 2)):
                        if ph == 1:
                            sot[c] = None
                            emit_ifg_mms(c, hi=8)
                            emit_sig(c, hi=8)
                            x_next(c)
                        else:
                            if r == 0 and c == 3:
                                continue
                            emit_cupd(c); emit_tanh(c); emit_h(c)
                elif SEQV == "lockstep":
                    emit_ifg_mms(A); emit_sig(A)
                    emit_ifg_mms(Bc); emit_sig(Bc)
                    emit_o_mms(A); emit_sigo(A); x_next(A)
                    emit_o_mms(Bc); emit_sigo(Bc); x_next(Bc)
                    emit_cupd(A); emit_tanh(A); emit_h(A)
                    emit_cupd(Bc); emit_tanh(Bc); emit_h(Bc)
                else:
                    # half-step offset: A runs step r in the first half of
                    # the round, B's step r-1 back-half completes, then B
                    # starts step r in the second half. The last step only
                    # needs c (the model output), so its o/tanh/h are dead.
                    last = r == T_FULL - 1
                    emit_ifg_mms(A); emit_sig(A)
                    if r > 0:
                        emit_cupd(Bc)          # B's step r-1 c-chain
                        emit_tanh(Bc)
                    if not last:
                        emit_o_mms(A); emit_sigo(A)
                    x_next(A)
                    if r > 0:
                        emit_h(Bc)
                    emit_cupd(A)
                    if not last:
                        emit_tanh(A)
                    emit_ifg_mms(Bc); emit_sig(Bc)
                    if not last:
                        emit_h(A)
                        emit_o_mms(Bc); emit_sigo(Bc)
                    x_next(Bc)

                # spread next iteration's prefetch: DMA launches first
                # (2/round), deferred transposes+copies later (1/round).
                for _ in range(2):
                    if dma_u:
                        dma_u.pop(0)()
                if xf_u and (r >= 4 if it == 0 else
                             (not dma_u and r % STEPS >= 4)):
                    xf_u.pop(0)()
                if r % STEPS == STEPS - 1 and embT_nxt is not None:
                    embT = embT_nxt
                    embT_nxt = embT_nx2
                    embT_nx2 = None

            def emit_epilogue(c):
                # dense epilogue: partial logits = (Wd_half)^T @ c
                dp = dps.tile([NUM_CLASSES, B], f32, tag=f"dp{c}",
                              name=f"dp{c}")
                for k in range(2):
                    nc.tensor.matmul(
                        out=dp[:], lhsT=wdT[:, k * 4:(k + 1) * 4],
                        rhs=cst[c][:, k * B:(k + 1) * B],
                        start=(k == 0), stop=(k == 1))
                ob = outp.tile([NUM_CLASSES, B], f32, tag=f"ob{c}",
                               name=f"ob{c}")
                nc.vector.tensor_copy(out=ob[:], in_=dp[:])
                nc.sync.dma_start(out=out_dram[c], in_=ob[:])

            if SEQV == "quad":
                emit_cupd(3)
                for c in range(CHAINS):
                    emit_epilogue(c)
            elif SEQV != "lockstep":
                emit_epilogue(A)
                emit_cupd(Bc)
                emit_epilogue(Bc)
            else:
                for c in range(CHAINS):
                    emit_epilogue(c)

    nc.compile()
    return nc


def _prep_core_inputs(core, x, emb_np, Wx, Wh, b, Wd):
    """Host-side prep: weight layout/scaling + gather index schedule."""
    d, s = core // 4, core % 4
    Wx = Wx.astype(np.float32).copy()
    Wh = Wh.astype(np.float32).copy()
    b = b.astype(np.float32).copy()
    # fold tanh(z_g) = 2*sigmoid(2 z_g) - 1: double the g-gate inputs.
    Wx[:, 512:768] *= 2.0
    b[512:768] *= 2.0
    Wh[:, 512:768] *= 2.0

    whx = np.empty((128, 24 * 128), np.float32)
    for m in range(8):
        for k in range(2):
            whx[:, (m * 3 + k) * 128:(m * 3 + k + 1) * 128] = \
                Wh[k * 128:(k + 1) * 128, m * 128:(m + 1) * 128]
        whx[:, (m * 3 + 2) * 128:(m * 3 + 3) * 128] = \
            Wx[:, m * 128:(m + 1) * 128]
    bb = np.repeat(b.reshape(8, 128).T[:, :, None], B, axis=2).reshape(128, GB)
    wdT = np.empty((128, 8), np.float32)
    for k in range(2):
        wdT[:, k * 4:(k + 1) * 4] = Wd[d * 256 + k * 128:
                                       d * 256 + (k + 1) * 128, :]

    it = np.arange(N_ITERS)[:, None, None]
    p = np.arange(128)[None, :, None]
    cj = np.arange(CHAINS * TPC)[None, None, :]
    chain, j = cj // TPC, cj % TPC
    s_local = j * (128 // B) + p // B
    jb = p % B
    t = it * STEPS + s_local
    if d == 1:
        t = (T_FULL - 1) - t
    row = s * 64 + chain * B + jb
    idx = x[row, t].astype(np.int32)          # [N_ITERS, 128, CHAINS*TPC]
    idxw = np.ascontiguousarray(
        idx.reshape(N_ITERS, 128, CHAINS, TPC).transpose(0, 2, 1, 3)
           .reshape(N_ITERS * CHAINS, 128, TPC))
    idx = np.ascontiguousarray(idx.transpose(1, 0, 2).reshape(128, -1))

    ins = {
        "emb": emb_np,
        "whxT": np.ascontiguousarray(whx.astype(W_NP)),
        "bbT": np.ascontiguousarray(bb.astype(W_NP)),
        "wdT": wdT,
        "identf": np.eye(128, dtype=np.float32),
        "identw": np.eye(128).astype(W_NP),
        "idx": idx,
    }
    if WIDE_GATHER:
        ins["idxw"] = idxw
    return ins


def kernel(x, train, embed_table, Wx_f, Wh_f, b_f, Wx_b, Wh_b, b_b, Wd, bd,
           **_unused):
    from concourse.bass_utils import run_bass_kernel_spmd

    x = np.asarray(x).astype(np.int64)
    emb_np = np.ascontiguousarray(
        np.asarray(embed_table, np.float32).astype(G_NP))
    Wd_np = np.asarray(Wd, np.float32)

    with_bias = bool(np.any(np.asarray(b_f)) or np.any(np.asarray(b_b)))
    key = ("nc", with_bias)
    if key not in _CACHE:
        _CACHE[key] = _build_program(with_bias)
    nc = _CACHE[key]

    in_maps = []
    for core in range(N_CORES):
        if core < 4:
            Wx, Wh, b = Wx_f, Wh_f, b_f
        else:
            Wx, Wh, b = Wx_b, Wh_b, b_b
        in_maps.append(_prep_core_inputs(
            core, x, emb_np, np.asarray(Wx), np.asarray(Wh), np.asarray(b),
            Wd_np))

    res = run_bass_kernel_spmd(nc, in_maps, list(range(N_CORES))).results

    logits = np.zeros((B_FULL, NUM_CLASSES), np.float32)
    for core in range(N_CORES):
        s = core % 4
        o = np.asarray(res[core]["out"], np.float32)  # [CHAINS, 4, B]
        for c in range(CHAINS):
            r0 = s * 64 + c * B
            logits[r0:r0 + B] += o[c].T
    logits += np.asarray(bd, np.float32)[None, :]
    return logits
